# revision 23
# baseline (speedup 1.0000x reference)
"""Trainium2 Bass kernel for nn_AttentionHead (B=4, n_ctx=4096, d_model=1024,
d_hidden=64, causal, scale=1/sqrt(d_model)).

Sharding: 8 cores = 4 batches x 2 balanced causal shards. Core (b, s) handles
the 2048 query rows in 64-row chunks with chunk%2 == s. Keys/x-columns are
permuted per core (my-parity chunks first within each 512-key ntile) so that
every core runs the IDENTICAL SPMD program:

  - slot j (0..3) = 512 queries = my chunks of ntiles 2j, 2j+1
  - slot j attends k-tiles t = 0..8(j+1)-1 (128 permuted keys each)
  - k-tiles t < 8j are fully open; t = 8j + r (r in 0..7) get an additive
    causal mask that depends only on (r, s) -> 8 mask tiles per core, sent
    as data.

Per-core pipeline (all matmuls in float32r: 1 cycle/row at N>=256):
  A: KT/VT = [Wk;Wov] @ xT (weights stationary, PSUM-accumulated over 8
     d_model chunks, biases folded in as K=1 rank-1 matmuls against a ones
     row); Q likewise from each ntile's first 256 columns (= my 4 chunks).
     V transposed to natural [k,64] layout via PE transpose, with an
     appended ones column so attn@[V,1] also yields the softmax denominator.
  B: S^T[k,q] = KT_tile^T @ QT_slot -> PSUM (two k-tiles paired per 2-bank
     PSUM tile); additive mask via identity matmul for diagonal tiles;
     exp((S+M)/32) on ACT over the pair -> SBUF (no row-max subtraction
     needed: |scores/32| <~ 1.5).
  C: O65 += V65_tile^T @ E (PSUM accumulate over k-tiles); row 64 = denom.
  D: y_tile = (O65_slice^T @ [Wo^T; bo]) * (1/den) -- den row makes the
     matmul add den*bo, so the per-partition reciprocal multiply yields
     O@Wo^T/den + bo exactly. Reciprocals come from 16 PE transposes of the
     denominator row into [128,16] + one DVE reciprocal.

DMA instruction count is kept low on purpose: each DMA holds the shared
HWDGE descriptor generator ~625ns, so x comes in as 16 [128,2048] loads and
y leaves as 16 [128,1024] stores; constants are single consolidated loads.
"""

import math

import numpy as np

D = 1024
H = 64
N = 4096
B = 4
CH = 64  # query chunk size (rows)
NT = 8  # ntiles of 512 keys
NEG = -1e10

_PROG = None  # cached compiled program


# ---------------------------------------------------------------- host layout


def _key_order(s: int) -> np.ndarray:
    order = []
    for n in range(NT):
        mine = [8 * n + t for t in range(8) if t % 2 == s]
        theirs = [8 * n + t for t in range(8) if t % 2 != s]
        for c in mine + theirs:
            order.extend(range(CH * c, CH * c + CH))
    return np.array(order)


def _masks(s: int) -> np.ndarray:
    ko = _key_order(s)
    qo = np.array([CH * c + i for c in range(s, 64, 2) for i in range(CH)])
    m = np.zeros((8, 128, 512), dtype=np.float32)
    for r in range(8):
        keys = ko[128 * r : 128 * (r + 1)]
        qs = qo[0:512]
        m[r] = np.where(keys[:, None] <= qs[None, :], 0.0, NEG)
    return m


# ---------------------------------------------------------------- bass program


def _build():
    import concourse.mybir as mybir
    import concourse.tile as tile
    from concourse import bacc

    f32 = mybir.dt.float32
    f32r = mybir.dt.float32r
    bf16 = mybir.dt.bfloat16

    nc = bacc.Bacc("TRN2", target_bir_lowering=False, debug=False, num_devices=8)

    xh = nc.dram_tensor("xh", [NT, 128, 8, 512], bf16, kind="ExternalInput").ap()
    wkv = nc.dram_tensor("wkv", [9, 128, 128], bf16, kind="ExternalInput").ap()
    wq = nc.dram_tensor("wq", [9, 128, 64], bf16, kind="ExternalInput").ap()
    wobo = nc.dram_tensor("wobo", [65, 1024], f32r, kind="ExternalInput").ap()
    masks = nc.dram_tensor("masks", [8, 128, 512], f32r, kind="ExternalInput").ap()
    ident = nc.dram_tensor("ident", [128, 128], f32r, kind="ExternalInput").ap()
    biases = nc.dram_tensor("biases", [128, 2], f32, kind="ExternalInput").ap()
    vones = nc.dram_tensor("vones", [128, 32, 1], f32r, kind="ExternalInput").ap()
    y = nc.dram_tensor("y", [2048, 1024], f32, kind="ExternalOutput").ap()

    Exp = mybir.ActivationFunctionType.Exp
    Identity = mybir.ActivationFunctionType.Identity
    mult = mybir.AluOpType.mult
    add_op = mybir.AluOpType.add
    scale = 1.0 / math.sqrt(D)

    with tile.TileContext(nc) as tc:
        with (
            tc.tile_pool(name="consts", bufs=1) as consts,
            tc.tile_pool(name="xp", bufs=3) as xpool,
            tc.tile_pool(name="ep", bufs=6) as epool,
            tc.tile_pool(name="yp", bufs=3) as ypool,
            tc.tile_pool(name="pkv", bufs=1, space="PSUM") as pkv,
            tc.tile_pool(name="pq", bufs=1, space="PSUM") as pq,
            tc.tile_pool(name="po", bufs=2, space="PSUM") as po,
            tc.tile_pool(name="ps", bufs=2, space="PSUM") as ps,
        ):
            # ---- constants (one DMA each)
            wkv_sb = consts.tile([128, 9 * 128], bf16)
            nc.gpsimd.dma_start(
                wkv_sb[:].rearrange("p (c f) -> p c f", c=9),
                wkv.rearrange("c p f -> p c f"),
            )
            wq_sb = consts.tile([128, 9 * 64], bf16)
            nc.gpsimd.dma_start(
                wq_sb[:].rearrange("p (c f) -> p c f", c=9),
                wq.rearrange("c p f -> p c f"),
            )
            id_sb = consts.tile([128, 128], f32r)
            nc.gpsimd.dma_start(id_sb[:], ident[:])
            bias_sb = consts.tile([128, 2], f32)  # col 0: [bk|bov], col 1: bq
            nc.gpsimd.dma_start(bias_sb[:], biases[:])

            kvt_sb = consts.tile([128, N], f32r)  # rows 0:64 KT, 64:128 VT
            qt_sb = consts.tile([H, 2048], f32r)
            v65_sb = consts.tile([128, 32 * 65], f32r)
            nc.gpsimd.dma_start(
                v65_sb[:].rearrange("p (t c) -> p t c", c=65)[:, :, 64:65], vones[:]
            )
            mask_sb = consts.tile([128, 8 * 512], f32r)
            nc.gpsimd.dma_start(
                mask_sb[:].rearrange("p (m f) -> p m f", m=8),
                masks.rearrange("m p f -> p m f"),
            )
            wobo_sb = consts.tile([65, 1024], f32r)
            nc.gpsimd.dma_start(wobo_sb[:], wobo[:])
            ot_sb = consts.tile([65, 2048], f32r)
            recip_sb = consts.tile([128, 16], f32)
            scratch_sb = consts.tile([1, 8], f32)

            # prewarm the ACT exp table while DMAs stream
            nc.scalar.activation(
                scratch_sb[:], id_sb[0:1, 0:8].bitcast(f32), Exp, bias=0.0, scale=1.0
            )

            # ---- interleaved emission: stage A ntiles, with slot j's
            # B/C/D emitted right after ntile 2j+1 so each engine's in-order
            # instruction stream matches data-readiness order.
            def emit_a(n):
                xn = xpool.tile([128, 4096], bf16, tag="x")
                xnv = xn[:].rearrange("p (c f) -> p c f", c=8)
                nc.sync.dma_start(xnv[:, 0:4, :], xh[n, :, 0:4, :])
                nc.sync.dma_start(xnv[:, 4:8, :], xh[n, :, 4:8, :])
                kvp = pkv.tile([128, 512], f32, tag="kv")
                qp = pq.tile([64, 256], f32, tag="q")
                for c in range(8):
                    nc.tensor.matmul(
                        kvp[:],
                        wkv_sb[:, 128 * c : 128 * (c + 1)],
                        xn[:, 512 * c : 512 * c + 512],
                        start=(c == 0),
                        stop=(c == 7),
                    )
                    nc.tensor.matmul(
                        qp[:],
                        wq_sb[:, 64 * c : 64 * (c + 1)],
                        xn[:, 512 * c : 512 * c + 256],
                        start=(c == 0),
                        stop=(c == 7),
                    )
                nc.scalar.activation(
                    kvt_sb[:, 512 * n : 512 * (n + 1)],
                    kvp[:],
                    Identity,
                    bias=bias_sb[:, 0:1],
                    scale=1.0,
                )
                nc.scalar.activation(
                    qt_sb[:, 256 * n : 256 * (n + 1)],
                    qp[:],
                    Identity,
                    bias=bias_sb[0:64, 1:2],
                    scale=1.0,
                )
                for t in range(4 * n, 4 * n + 4):
                    vp = pq.tile([128, 64], f32r, tag="q")
                    nc.tensor.transpose(
                        vp[:],
                        kvt_sb[64:128, 128 * t : 128 * (t + 1)],
                        id_sb[64:128, 64:128],
                    )
                    nc.vector.tensor_copy(v65_sb[:, 65 * t : 65 * t + 64], vp[:])

            def emit_d(i):
                ys = ypool.tile([128, 1024], f32, tag="y")
                yp = ps.tile([128, 1024], f32, tag="s")
                for d in range(2):
                    nc.tensor.matmul(
                        yp[:, 512 * d : 512 * (d + 1)],
                        ot_sb[:, 128 * i : 128 * (i + 1)],
                        wobo_sb[:, 512 * d : 512 * (d + 1)],
                        start=True,
                        stop=True,
                    )
                if i % 2 == 0:
                    nc.vector.tensor_scalar(
                        out=ys[:],
                        in0=yp[:],
                        scalar1=recip_sb[:, i : i + 1],
                        scalar2=None,
                        op0=mult,
                    )
                else:
                    nc.scalar.activation(
                        ys[:],
                        yp[:],
                        Identity,
                        bias=0.0,
                        scale=recip_sb[:, i : i + 1],
                    )
                nc.gpsimd.dma_start(y[128 * i : 128 * (i + 1), :], ys[:])

            pending_d = []

            def emit_bc(j):
                op_ = po.tile([65, 512], f32, tag="o")
                nk = 8 * (j + 1)
                def emit_c(t0, et):
                    for h in range(2):
                        t = t0 + h
                        nc.tensor.matmul(
                            op_[:],
                            v65_sb[:, 65 * t : 65 * (t + 1)],
                            et[:, 512 * h : 512 * (h + 1)],
                            start=(t == 0),
                            stop=(t == nk - 1),
                        )

                # software-pipelined: C trails one pair behind B/exp so the
                # PE stream never waits on the exp of the pair it just fed
                prev = None
                for t0 in range(0, nk, 2):
                    if pending_d and t0 % 4 == 2:
                        emit_d(pending_d.pop(0))
                    sp = ps.tile([128, 1024], f32, tag="s")
                    for h in range(2):
                        t = t0 + h
                        rr = t - 8 * j
                        nc.tensor.matmul(
                            sp[:, 512 * h : 512 * (h + 1)],
                            kvt_sb[0:64, 128 * t : 128 * (t + 1)],
                            qt_sb[:, 512 * j : 512 * (j + 1)],
                            start=True,
                            stop=(rr < 0),
                        )
                        if rr >= 0:
                            nc.tensor.matmul(
                                sp[:, 512 * h : 512 * (h + 1)],
                                id_sb[:],
                                mask_sb[:, 512 * rr : 512 * (rr + 1)],
                                start=False,
                                stop=True,
                            )
                    et = epool.tile([128, 1024], f32r, tag="e")
                    nc.scalar.activation(et[:], sp[:], Exp, bias=0.0, scale=scale)
                    if prev is not None:
                        emit_c(*prev)
                    prev = (t0, et)
                emit_c(*prev)
                nc.vector.tensor_copy(ot_sb[:, 512 * j : 512 * (j + 1)], op_[:])
                rp = pq.tile([128, 4], f32, tag="q")
                for ii in range(4):
                    i = 4 * j + ii
                    nc.tensor.transpose(
                        rp[:, ii : ii + 1],
                        ot_sb[64:65, 128 * i : 128 * (i + 1)].bitcast(f32),
                        id_sb[64:65, 64:65].bitcast(f32),
                    )
                nc.vector.reciprocal(recip_sb[:, 4 * j : 4 * j + 4], rp[:])
                pending_d.extend(range(4 * j, 4 * j + 4))

            for n in range(NT):
                emit_a(n)
                if n % 2 == 1:
                    emit_bc((n - 1) // 2)

            for i in pending_d:
                emit_d(i)

    nc.compile()
    return nc


def _get_prog():
    global _PROG
    if _PROG is None:
        _PROG = _build()
    return _PROG


# ---------------------------------------------------------------- entry point


def _xh(xb, korder):
    """[ntile, partition, chunk, 512] bf16 layout of x[b][korder].T."""
    import ml_dtypes

    xt = xb[korder].T  # [1024, 4096]
    return np.ascontiguousarray(
        xt.reshape(8, 128, 8, 512).transpose(2, 1, 0, 3).astype(ml_dtypes.bfloat16)
    )


def kernel(x, Wq, bq, Wk, bk, Wov, bov, Wo, bo, _trace=False):
    from concourse import bass_utils

    x = np.ascontiguousarray(np.asarray(x, dtype=np.float32))
    Wq = np.asarray(Wq, dtype=np.float32)
    bq = np.asarray(bq, dtype=np.float32)
    Wk = np.asarray(Wk, dtype=np.float32)
    bk = np.asarray(bk, dtype=np.float32)
    Wov = np.asarray(Wov, dtype=np.float32)
    bov = np.asarray(bov, dtype=np.float32)
    Wo = np.asarray(Wo, dtype=np.float32)
    bo = np.asarray(bo, dtype=np.float32)

    nc = _get_prog()

    wkv_arr = np.zeros((9, 128, 128), dtype=np.float32)
    wkv_t = np.concatenate([Wk, Wov], axis=0).T  # [1024, 128]
    for c in range(8):
        wkv_arr[c] = wkv_t[128 * c : 128 * (c + 1)]
    wkv_arr[8][0] = np.concatenate([bk, bov])

    wq_arr = np.zeros((9, 128, 64), dtype=np.float32)
    wq_t = Wq.T  # [1024, 64]
    for c in range(8):
        wq_arr[c] = wq_t[128 * c : 128 * (c + 1)]
    wq_arr[8][0] = bq

    import ml_dtypes

    wobo_arr = np.concatenate([Wo.T, bo[None, :]], axis=0)  # [65, 1024]
    wkv_arr = wkv_arr.astype(ml_dtypes.bfloat16)
    wq_arr = wq_arr.astype(ml_dtypes.bfloat16)
    biases_arr = np.zeros((128, 2), dtype=np.float32)
    biases_arr[:, 0] = np.concatenate([bk, bov])
    biases_arr[0:64, 1] = bq
    ident_arr = np.eye(128, dtype=np.float32)
    masks_s = [_masks(0), _masks(1)]
    korder_s = [_key_order(0), _key_order(1)]

    in_maps = []
    for core in range(8):
        b, s = divmod(core, 2)
        in_maps.append(
            {
                "xh": _xh(x[b], korder_s[s]),
                "wkv": wkv_arr,
                "wq": wq_arr,
                "wobo": wobo_arr,
                "masks": masks_s[s],
                "ident": ident_arr,
                "biases": biases_arr,
                "vones": np.ones((128, 32, 1), dtype=np.float32),
            }
        )

    res = bass_utils.run_bass_kernel_spmd(
        nc, in_maps, core_ids=list(range(8)), trace=_trace
    )

    y = np.empty((B, N, D), dtype=np.float32)
    for core in range(8):
        b, s = divmod(core, 2)
        y[b].reshape(64, CH, D)[s::2] = res.results[core]["y"].reshape(32, CH, D)
    return y
